# revision 9
# baseline (speedup 1.0000x reference)
"""BloomAttention (B=1, S=2048, H=4096, NH=32) on 8 Trainium2 cores.

Megatron-style tensor parallelism over heads: each core owns 4 heads.
 - QKV projection: column-parallel (each core computes its heads' Q/K/V)
 - attention: fully local per core (head-parallel)
 - dense projection: row-parallel -> per-core partial outputs, summed on host

All matmuls run in float32r (TF32-like, full PE speed at moving dim >=256).
Layouts keep the contraction dim on SBUF partitions:
   hiddenT [H, S], w_qkvT [H, 3*512], QT/KT/VT per head [128, S],
   probsT [keys, q], ctxT [128, S], w_denseT [512, H].
Causal structure is exploited by truncating each 128-query block's key range;
the diagonal 128x128 block is masked with a host-provided additive tile.
"""
import math
import numpy as np
from contextlib import ExitStack

import concourse.bacc as bacc
import concourse.bass as bass
import concourse.mybir as mybir
import concourse.tile as tile
from concourse.bass_utils import run_bass_kernel_spmd

# problem dims (hardcoded per contract)
B, S, H, NH = 1, 2048, 4096, 32
HD = H // NH            # 128
NCORES = 8
HPC = NH // NCORES      # 4 heads per core
DPC = HPC * HD          # 512 features per core
FC = 3 * HPC            # 12 feature chunks of 128 in QKV output
INV_NORM = 1.0 / math.sqrt(HD)
NEG = float(np.finfo(np.float32).min)
P = 128
QB = S // P             # 16 query blocks
F32 = mybir.dt.float32
F32R = mybir.dt.float32r

_CACHE = {}


def _build(kNq):
    """Build the SPMD program for one core. kNq[qb] = key columns needed for
    query block qb (multiple of 128). Returns compiled Bacc."""
    nc = bacc.Bacc("TRN2", target_bir_lowering=False, debug=False,
                   num_devices=NCORES)

    hT = nc.dram_tensor("hT", [H, S], F32R, kind="ExternalInput")
    wqkvT = nc.dram_tensor("wqkvT", [H, FC * P], F32R, kind="ExternalInput")
    bqkv = nc.dram_tensor("bqkv", [FC, P], F32, kind="ExternalInput")
    alibi_t = nc.dram_tensor("alibi_c", [HPC, S], F32, kind="ExternalInput")
    tri_t = nc.dram_tensor("tri", [QB, P, P], F32, kind="ExternalInput")
    ident_t = nc.dram_tensor("ident", [P, P], F32R, kind="ExternalInput")
    wdT = nc.dram_tensor("wdT", [DPC, H], F32R, kind="ExternalInput")
    ctx_sp = nc.dram_tensor("ctx_spill", [HPC, P, S], F32R)
    out_t = nc.dram_tensor("out_part", [S, H], F32, kind="ExternalOutput")

    KP = 8                      # contraction panels of 512 rows
    JP = H // KP // P           # 4 h-chunks per panel

    with tile.TileContext(nc) as tc, ExitStack() as top:
        singles = top.enter_context(tc.tile_pool(name="singles", bufs=1))
        ph12 = top.enter_context(ExitStack())
        qkv_pool = ph12.enter_context(tc.tile_pool(name="qkv", bufs=1))
        # persistent QT/KT/VT tiles: [comp][head] -> [128, S]
        qkv_tiles = [[qkv_pool.tile([P, S], F32R, tag=f"qkv_{c}_{h}",
                                    name=f"qkv_{c}_{h}")
                      for h in range(HPC)] for c in range(3)]
        ident_sb = singles.tile([P, P], F32R, tag="ident")
        nc.sync.dma_start(out=ident_sb, in_=ident_t[:, :])
        bias_sb = singles.tile([P, FC], F32, tag="bias")
        nc.sync.dma_start(
            out=bias_sb,
            in_=bass.AP(tensor=bqkv, offset=0, ap=[[1, P], [P, FC]]))

        # ---------------- phase 1: QKV projection ----------------
        with ExitStack() as ph1:
            hid_pool = ph1.enter_context(tc.tile_pool(name="hid", bufs=2))
            wq_pool = ph1.enter_context(tc.tile_pool(name="wq", bufs=3))
            ps1 = ph1.enter_context(
                tc.tile_pool(name="ps1", bufs=8, space="PSUM"))
            for kp in range(KP):
                hp = hid_pool.tile([P, JP, S], F32R, tag="hp")
                nc.sync.dma_start(
                    out=hp,
                    in_=hT[kp * JP * P:(kp + 1) * JP * P, :].rearrange(
                        "(j p) s -> p j s", p=P))
                for fc in range(FC):
                    wt = wq_pool.tile([P, JP, P], F32R, tag="wt")
                    nc.sync.dma_start(
                        out=wt,
                        in_=wqkvT[kp * JP * P:(kp + 1) * JP * P,
                                  fc * P:(fc + 1) * P].rearrange(
                                      "(j p) f -> p j f", p=P))
                    comp, head = fc // HPC, fc % HPC
                    dest = qkv_tiles[comp][head]
                    for sb4 in range(S // 512):
                        ps = ps1.tile([P, 512], F32, tag="ps1")
                        for j in range(JP):
                            nc.tensor.matmul(
                                ps, wt[:, j, :],
                                hp[:, j, sb4 * 512:(sb4 + 1) * 512],
                                start=(j == 0), stop=(j == JP - 1))
                        dsl = dest[:, sb4 * 512:(sb4 + 1) * 512]
                        if kp == 0:
                            nc.scalar.activation(
                                out=dsl, in_=ps,
                                func=mybir.ActivationFunctionType.Identity,
                                bias=bias_sb[:, fc:fc + 1], scale=1.0)
                        else:
                            nc.vector.tensor_add(
                                out=dsl, in0=ps, in1=dsl)

        # ---------------- phase 2: attention ----------------
        with ExitStack() as ph2:
            al_pool = ph2.enter_context(tc.tile_pool(name="alibi", bufs=1))
            vn_pool = ph2.enter_context(tc.tile_pool(name="vnat", bufs=2))
            pr_pool = ph2.enter_context(tc.tile_pool(name="prow", bufs=2))
            pq_pool = ph2.enter_context(tc.tile_pool(name="pquad", bufs=5))
            tri_pool = ph2.enter_context(tc.tile_pool(name="tri", bufs=2))
            sm_pool = ph2.enter_context(tc.tile_pool(name="small", bufs=8))
            cs_pool = ph2.enter_context(tc.tile_pool(name="ctxstage", bufs=2))
            ps_sc = ph2.enter_context(
                tc.tile_pool(name="ps_sc", bufs=4, space="PSUM"))
            ps_st = ph2.enter_context(
                tc.tile_pool(name="ps_st", bufs=2, space="PSUM"))
            ps_cx = ph2.enter_context(
                tc.tile_pool(name="ps_cx", bufs=2, space="PSUM"))

            NG = QB // 4        # 4 query groups of 512
            for h in range(HPC):
                qt, kt, vt = (qkv_tiles[0][h], qkv_tiles[1][h],
                              qkv_tiles[2][h])
                alb = al_pool.tile([P, S], F32, tag="alb")
                nc.gpsimd.dma_start(
                    out=alb,
                    in_=bass.AP(tensor=alibi_t, offset=h * S,
                                ap=[[0, P], [1, S]]))
                # V -> natural [keys, HD] layout via PE transpose
                vnat = vn_pool.tile([P, S // P, P], F32R, tag="vnat")
                for t4 in range(S // P // 4):
                    stg = ps_st.tile([P, 4, P], F32R, tag="stg")
                    for j in range(4):
                        nc.tensor.transpose(
                            stg[:, j, :],
                            vt[:, (4 * t4 + j) * P:(4 * t4 + j + 1) * P],
                            ident_sb)
                    nc.scalar.copy(out=vnat[:, 4 * t4:4 * t4 + 4, :], in_=stg)

                for qg in range(NG):
                    kns = [kNq[4 * qg + i] for i in range(4)]
                    ntile = max(kns) // P
                    nquad = (ntile + 3) // 4
                    quads = [pq_pool.tile([P, 4, 512], F32R, tag="pquad",
                                          name=f"pq_{h}_{qg}_{a}")
                             for a in range(nquad)]

                    for qbl in range(4):
                        qb = 4 * qg + qbl
                        kN = kNq[qb]
                        nch = (kN + 511) // 512
                        prow = pr_pool.tile([P, S], F32R, tag="prow")
                        nmax = sm_pool.tile([P, 4], F32, tag="nmax")
                        nbias = sm_pool.tile([P, 1], F32, tag="nbias")
                        sums = sm_pool.tile([P, 4], F32, tag="sums")
                        tot = sm_pool.tile([P, 1], F32, tag="tot")
                        rinv = sm_pool.tile([P, 1], F32, tag="rinv")
                        tri_sb = tri_pool.tile([P, P], F32, tag="tri")
                        nc.sync.dma_start(out=tri_sb, in_=tri_t[qb])
                        chunks = []
                        for kc in range(nch):
                            N = min(512, kN - 512 * kc)
                            ps = ps_sc.tile([P, 512], F32, tag="ps_sc")
                            chunks.append((ps, N))
                            nc.tensor.matmul(
                                ps[:, :N], qt[:, qb * P:(qb + 1) * P],
                                kt[:, kc * 512:kc * 512 + N],
                                start=True, stop=True)
                            nc.vector.tensor_add(
                                out=ps[:, :N], in0=ps[:, :N],
                                in1=alb[:, kc * 512:kc * 512 + N])
                            if kc == nch - 1:
                                nc.vector.tensor_add(
                                    out=ps[:, N - P:N], in0=ps[:, N - P:N],
                                    in1=tri_sb)
                            nc.vector.tensor_reduce(
                                out=nmax[:, kc:kc + 1], in_=ps[:, :N],
                                op=mybir.AluOpType.max,
                                axis=mybir.AxisListType.X)
                        nc.vector.tensor_reduce(
                            out=nbias, in_=nmax[:, :nch],
                            op=mybir.AluOpType.max,
                            axis=mybir.AxisListType.X, negate=True)
                        for kc, (ps, N) in enumerate(chunks):
                            nc.scalar.activation(
                                out=prow[:, kc * 512:kc * 512 + N],
                                in_=ps[:, :N],
                                func=mybir.ActivationFunctionType.Exp,
                                bias=nbias, scale=1.0,
                                accum_out=sums[:, kc:kc + 1])
                        nc.vector.tensor_reduce(
                            out=tot, in_=sums[:, :nch],
                            op=mybir.AluOpType.add, axis=mybir.AxisListType.X)
                        nc.vector.reciprocal(out=rinv, in_=tot)
                        nc.vector.tensor_scalar_mul(
                            out=prow[:, :kN], in0=prow[:, :kN], scalar1=rinv)
                        # transpose probs into key-major quads
                        # quad tile layout: [P, tile_in_quad(4), q(512)]
                        ntile_q = kN // P
                        t = 0
                        while t < ntile_q:
                            cnt = min(4, ntile_q - t)
                            stg = ps_st.tile([P, 4, P], F32R, tag="stg")
                            for j in range(cnt):
                                nc.tensor.transpose(
                                    stg[:, j, :],
                                    prow[:, (t + j) * P:(t + j + 1) * P],
                                    ident_sb)
                            nc.scalar.copy(
                                out=quads[t // 4][:, :cnt,
                                                  qbl * P:(qbl + 1) * P],
                                in_=stg[:, :cnt, :])
                            t += cnt
                    # PV: ctxT[128, 512] accumulate over key tiles.
                    # Ragged key tiles only contribute to the query sub-blocks
                    # that cover them -> accumulate into a column slice; tile 0
                    # is covered by every sub-block, so start=True initializes
                    # the full bank.
                    tiles_per_qbl = [kns[i] // P for i in range(4)]
                    cps = ps_cx.tile([P, 512], F32, tag="ps_cx")
                    for t in range(ntile):
                        q0 = P * min(i for i in range(4)
                                     if tiles_per_qbl[i] > t)
                        nc.tensor.matmul(
                            cps[:, q0:], vnat[:, t, :],
                            quads[t // 4][:, t % 4, q0:],
                            start=(t == 0), stop=(t == ntile - 1))
                    cst = cs_pool.tile([P, 512], F32R, tag="cst")
                    nc.scalar.copy(out=cst, in_=cps)
                    nc.sync.dma_start(
                        out=ctx_sp[h, :, qg * 512:(qg + 1) * 512], in_=cst)

        ph12.close()  # free QKV + attention SBUF before dense phase

        # ---------------- phase 3: dense projection ----------------
        with ExitStack() as ph3:
            cx_pool = ph3.enter_context(tc.tile_pool(name="cx", bufs=1))
            wd_pool = ph3.enter_context(tc.tile_pool(name="wd", bufs=1))
            st_pool = ph3.enter_context(tc.tile_pool(name="ostage", bufs=2))
            ps3 = ph3.enter_context(
                tc.tile_pool(name="ps3", bufs=8, space="PSUM"))
            ctx_sb = [cx_pool.tile([P, S], F32R, tag=f"ctx_{hh}", name=f"ctx_{hh}")
                      for hh in range(HPC)]
            for hh in range(HPC):
                nc.sync.dma_start(out=ctx_sb[hh], in_=ctx_sp[hh])
            wdt = wd_pool.tile([P, HPC, H], F32R, tag="wdt")
            for hh in range(HPC):
                nc.sync.dma_start(
                    out=wdt[:, hh, :],
                    in_=wdT[hh * P:(hh + 1) * P, :])
            for qb in range(QB):
                pss = [ps3.tile([P, 512], F32, tag="ps3", name=f"ps3_{qb}_{i}")
                       for i in range(8)]
                for dc in range(HPC):
                    for oc in range(8):
                        nc.tensor.matmul(
                            pss[oc], ctx_sb[dc][:, qb * P:(qb + 1) * P],
                            wdt[:, dc, oc * 512:(oc + 1) * 512],
                            start=(dc == 0), stop=(dc == HPC - 1))
                stage = st_pool.tile([P, H], F32, tag="ostage")
                for oc in range(8):
                    nc.any.tensor_copy(
                        out=stage[:, oc * 512:(oc + 1) * 512], in_=pss[oc])
                nc.sync.dma_start(
                    out=out_t[qb * P:(qb + 1) * P, :], in_=stage)

    nc.compile()
    return nc


def _host_prep(hidden_states, alibi, attention_mask, w_qkv, b_qkv, w_dense):
    """Returns (kNq, in_maps) for the 8 cores."""
    hidden = np.asarray(hidden_states, np.float32).reshape(S, H)
    mask = np.asarray(attention_mask).reshape(S, S)
    alibi = np.asarray(alibi, np.float32).reshape(NH, S)
    w_qkv = np.asarray(w_qkv, np.float32)
    b_qkv = np.asarray(b_qkv, np.float32)
    w_dense = np.asarray(w_dense, np.float32)

    allowed = ~mask
    assert allowed.any(axis=1).all(), "fully-masked row"
    limit = S - np.argmax(allowed[:, ::-1], axis=1)      # last allowed + 1
    recon = np.arange(S)[None, :] >= limit[:, None]
    if not np.array_equal(mask, recon):
        raise NotImplementedError("mask is not suffix-structured")
    kNq, tri = [], np.zeros((QB, P, P), np.float32)
    col = np.arange(S)
    for qb in range(QB):
        lb = limit[qb * P:(qb + 1) * P]
        kN = int(math.ceil(lb.max() / P) * P)
        if lb.min() < kN - P:
            raise NotImplementedError("mask boundary spans >128 cols in block")
        kNq.append(kN)
        cc = col[kN - P:kN]
        tri[qb] = np.where(cc[None, :] >= lb[:, None], NEG, 0.0)
    if any(kNq[i] > kNq[i + 1] for i in range(QB - 1)):
        raise NotImplementedError("non-monotone key ranges")

    hT = np.ascontiguousarray(hidden.T)                  # [H, S]
    wr = w_qkv.reshape(NH, 3, HD, H)
    br = b_qkv.reshape(NH, 3, HD)
    ident = np.eye(P, dtype=np.float32)

    in_maps = []
    for c in range(NCORES):
        hs = slice(HPC * c, HPC * (c + 1))
        Wq = wr[hs, 0].reshape(DPC, H) * INV_NORM
        Wk = wr[hs, 1].reshape(DPC, H)
        Wv = wr[hs, 2].reshape(DPC, H)
        wqkvT_c = np.ascontiguousarray(
            np.concatenate([Wq, Wk, Wv], axis=0).T)      # [H, 1536]
        bq = br[hs, 0].reshape(-1) * INV_NORM
        bk = br[hs, 1].reshape(-1)
        bv = br[hs, 2].reshape(-1)
        bqkv_c = np.concatenate([bq, bk, bv]).reshape(FC, P)
        wdT_c = np.ascontiguousarray(
            w_dense[:, DPC * c:DPC * (c + 1)].T)         # [512, H]
        in_maps.append({
            "hT": hT, "wqkvT": wqkvT_c, "bqkv": bqkv_c,
            "alibi_c": np.ascontiguousarray(alibi[hs]),
            "tri": tri, "ident": ident, "wdT": wdT_c,
        })
    return tuple(kNq), in_maps


def kernel(hidden_states, residual, alibi, attention_mask,
           w_qkv, b_qkv, w_dense, b_dense):
    kNq, in_maps = _host_prep(hidden_states, alibi, attention_mask,
                              w_qkv, b_qkv, w_dense)
    if kNq not in _CACHE:
        _CACHE[kNq] = _build(kNq)
    nc = _CACHE[kNq]
    res = run_bass_kernel_spmd(nc, in_maps, list(range(NCORES)))
    acc = res.results[0]["out_part"].astype(np.float64)
    for c in range(1, NCORES):
        acc += res.results[c]["out_part"]
    out = acc.astype(np.float32) + np.asarray(b_dense, np.float32)[None, :]
    out = out + np.asarray(residual, np.float32).reshape(S, H)
    return out.reshape(B, S, H).astype(np.float32)


# revision 13
# speedup vs baseline: 18034.4035x; 18034.4035x over previous
"""BloomAttention (B=1, S=2048, H=4096, NH=32) on 8 Trainium2 cores.

Megatron-style tensor parallelism over heads: each core owns 4 heads.
 - QKV projection: column-parallel (each core computes its heads' Q/K/V)
 - attention: fully local per core (head-parallel)
 - dense projection: row-parallel -> per-core partial outputs, summed on host

All matmuls run in float32r (TF32-like, full PE speed at moving dim >=256).
Layouts keep the contraction dim on SBUF partitions:
   hiddenT [H, S], w_qkvT [H, 3*512], QT/KT/VT per head [128, S],
   probsT [keys, q], ctxT [128, S], w_denseT [512, H].
Causal structure is exploited by truncating each 128-query block's key range;
the diagonal 128x128 block is masked with a host-provided additive tile.
"""
import math
import numpy as np
from contextlib import ExitStack

import concourse.bacc as bacc
import concourse.bass as bass
import concourse.mybir as mybir
import concourse.tile as tile
from concourse.bass_utils import run_bass_kernel_spmd

# problem dims (hardcoded per contract)
B, S, H, NH = 1, 2048, 4096, 32
HD = H // NH            # 128
NCORES = 8
HPC = NH // NCORES      # 4 heads per core
DPC = HPC * HD          # 512 features per core
FC = 3 * HPC            # 12 feature chunks of 128 in QKV output
INV_NORM = 1.0 / math.sqrt(HD)
NEG = float(np.finfo(np.float32).min)
P = 128
QB = S // P             # 16 query blocks
F32 = mybir.dt.float32
F32R = mybir.dt.float32r

_CACHE = {}


def _build(kNq):
    """Build the SPMD program for one core. kNq[qb] = key columns needed for
    query block qb (multiple of 128). Returns compiled Bacc."""
    nc = bacc.Bacc("TRN2", target_bir_lowering=False, debug=False,
                   num_devices=NCORES)

    hT = nc.dram_tensor("hT", [H, S], F32R, kind="ExternalInput")
    wqkvT = nc.dram_tensor("wqkvT", [H, FC * P], F32R, kind="ExternalInput")
    bqkv = nc.dram_tensor("bqkv", [FC, P], F32, kind="ExternalInput")
    alibi_t = nc.dram_tensor("alibi_c", [HPC, S], F32, kind="ExternalInput")
    tri_t = nc.dram_tensor("tri", [QB, P, P], F32, kind="ExternalInput")
    ident_t = nc.dram_tensor("ident", [P, P], F32R, kind="ExternalInput")
    wdT = nc.dram_tensor("wdT", [DPC, H], F32R, kind="ExternalInput")
    ctx_sp = nc.dram_tensor("ctx_spill", [HPC, P, S], F32R)
    out_t = nc.dram_tensor("out_part", [S, H], F32, kind="ExternalOutput")

    KP = 8                      # contraction panels of 512 rows
    JP = H // KP // P           # 4 h-chunks per panel

    with tile.TileContext(nc) as tc, ExitStack() as top:
        singles = top.enter_context(tc.tile_pool(name="singles", bufs=1))
        ph12 = top.enter_context(ExitStack())
        qkv_pool = ph12.enter_context(tc.tile_pool(name="qkv", bufs=1))
        # persistent QT/KT/VT tiles: [comp][head] -> [128, S]
        qkv_tiles = [[qkv_pool.tile([P, S], F32R, tag=f"qkv_{c}_{h}",
                                    name=f"qkv_{c}_{h}")
                      for h in range(HPC)] for c in range(3)]
        ident_sb = singles.tile([P, P], F32R, tag="ident")
        nc.sync.dma_start(out=ident_sb, in_=ident_t[:, :])
        bias_sb = singles.tile([P, FC], F32, tag="bias")
        nc.sync.dma_start(
            out=bias_sb,
            in_=bass.AP(tensor=bqkv, offset=0, ap=[[1, P], [P, FC]]))

        # ---------------- phase 1: QKV projection ----------------
        with ExitStack() as ph1:
            hid_pool = ph1.enter_context(tc.tile_pool(name="hid", bufs=2))
            wq_pool = ph1.enter_context(tc.tile_pool(name="wq", bufs=3))
            ps1 = ph1.enter_context(
                tc.tile_pool(name="ps1", bufs=8, space="PSUM"))
            for kp in range(KP):
                hp = hid_pool.tile([P, JP, S], F32R, tag="hp")
                for j in range(JP):
                    r0 = (kp * JP + j) * P
                    nc.sync.dma_start(out=hp[:, j, :], in_=hT[r0:r0 + P, :])
                for fc in range(FC):
                    wt = wq_pool.tile([P, JP, P], F32R, tag="wt")
                    nc.sync.dma_start(
                        out=wt,
                        in_=wqkvT[kp * JP * P:(kp + 1) * JP * P,
                                  fc * P:(fc + 1) * P].rearrange(
                                      "(j p) f -> p j f", p=P))
                    comp, head = fc // HPC, fc % HPC
                    dest = qkv_tiles[comp][head]
                    for sb4 in range(S // 512):
                        ps = ps1.tile([P, 512], F32, tag="ps1")
                        for j in range(JP):
                            nc.tensor.matmul(
                                ps, wt[:, j, :],
                                hp[:, j, sb4 * 512:(sb4 + 1) * 512],
                                start=(j == 0), stop=(j == JP - 1))
                        dsl = dest[:, sb4 * 512:(sb4 + 1) * 512]
                        if kp == 0:
                            nc.scalar.activation(
                                out=dsl, in_=ps,
                                func=mybir.ActivationFunctionType.Identity,
                                bias=bias_sb[:, fc:fc + 1], scale=1.0)
                        else:
                            nc.vector.tensor_add(
                                out=dsl, in0=ps, in1=dsl)

        # ---------------- phase 2: attention ----------------
        with ExitStack() as ph2:
            al_pool = ph2.enter_context(tc.tile_pool(name="alibi", bufs=1))
            vn_pool = ph2.enter_context(tc.tile_pool(name="vnat", bufs=1))
            sr_pool = ph2.enter_context(tc.tile_pool(name="srow", bufs=2))
            pr_pool = ph2.enter_context(tc.tile_pool(name="prow", bufs=2))
            pq_pool = ph2.enter_context(tc.tile_pool(name="pquad", bufs=5))
            tri_pool = ph2.enter_context(tc.tile_pool(name="tri", bufs=2))
            sm_pool = ph2.enter_context(tc.tile_pool(name="small", bufs=8))
            cs_pool = ph2.enter_context(tc.tile_pool(name="ctxstage", bufs=2))
            ps_sc = ph2.enter_context(
                tc.tile_pool(name="ps_sc", bufs=3, space="PSUM"))
            ps_st = ph2.enter_context(
                tc.tile_pool(name="ps_st", bufs=3, space="PSUM"))
            ps_cx = ph2.enter_context(
                tc.tile_pool(name="ps_cx", bufs=2, space="PSUM"))

            NG = QB // 4        # 4 query groups of 512
            for h in range(HPC):
                qt, kt, vt = (qkv_tiles[0][h], qkv_tiles[1][h],
                              qkv_tiles[2][h])
                alb = al_pool.tile([P, S], F32, tag="alb")
                nc.gpsimd.dma_start(
                    out=alb,
                    in_=bass.AP(tensor=alibi_t, offset=h * S,
                                ap=[[0, P], [1, S]]))
                # V -> natural [keys, HD] layout via PE transpose
                vnat = vn_pool.tile([P, S // P, P], F32R, tag="vnat")
                for t4 in range(S // P // 4):
                    stg = ps_st.tile([P, 4, P], F32R, tag="stg")
                    for j in range(4):
                        nc.tensor.transpose(
                            stg[:, j, :],
                            vt[:, (4 * t4 + j) * P:(4 * t4 + j + 1) * P],
                            ident_sb)
                    nc.scalar.copy(out=vnat[:, 4 * t4:4 * t4 + 4, :], in_=stg)

                for qg in range(NG):
                    kns = [kNq[4 * qg + i] for i in range(4)]
                    ntile = max(kns) // P
                    nquad = (ntile + 3) // 4
                    quads = [pq_pool.tile([P, 4, 512], F32R, tag="pquad",
                                          name=f"pq_{h}_{qg}_{a}")
                             for a in range(nquad)]

                    for qbl in range(4):
                        qb = 4 * qg + qbl
                        kN = kNq[qb]
                        nch = (kN + 511) // 512
                        srow = sr_pool.tile([P, S], F32, tag="srow")
                        prow = pr_pool.tile([P, S], F32R, tag="prow")
                        nbias = sm_pool.tile([P, 1], F32, tag="nbias")
                        tot = sm_pool.tile([P, 1], F32, tag="tot")
                        rinv = sm_pool.tile([P, 1], F32, tag="rinv")
                        tri_sb = tri_pool.tile([P, P], F32, tag="tri")
                        nc.sync.dma_start(out=tri_sb, in_=tri_t[qb])
                        for kc in range(nch):
                            N = min(512, kN - 512 * kc)
                            ps = ps_sc.tile([P, 512], F32, tag="ps_sc")
                            nc.tensor.matmul(
                                ps[:, :N], qt[:, qb * P:(qb + 1) * P],
                                kt[:, kc * 512:kc * 512 + N],
                                start=True, stop=True)
                            # scores + alibi -> SBUF (frees the PSUM bank)
                            nc.vector.tensor_add(
                                out=srow[:, kc * 512:kc * 512 + N],
                                in0=ps[:, :N],
                                in1=alb[:, kc * 512:kc * 512 + N])
                        nc.vector.tensor_add(
                            out=srow[:, kN - P:kN], in0=srow[:, kN - P:kN],
                            in1=tri_sb)
                        nc.vector.tensor_reduce(
                            out=nbias, in_=srow[:, :kN],
                            op=mybir.AluOpType.max,
                            axis=mybir.AxisListType.X, negate=True)
                        nc.scalar.activation(
                            out=prow[:, :kN], in_=srow[:, :kN],
                            func=mybir.ActivationFunctionType.Exp,
                            bias=nbias, scale=1.0, accum_out=tot)
                        nc.vector.reciprocal(out=rinv, in_=tot)
                        nc.vector.tensor_scalar_mul(
                            out=prow[:, :kN], in0=prow[:, :kN], scalar1=rinv)
                        # transpose probs into key-major quads
                        # quad tile layout: [P, tile_in_quad(4), q(512)]
                        ntile_q = kN // P
                        t = 0
                        while t < ntile_q:
                            cnt = min(4, ntile_q - t)
                            stg = ps_st.tile([P, 4, P], F32R, tag="stg")
                            for j in range(cnt):
                                nc.tensor.transpose(
                                    stg[:, j, :],
                                    prow[:, (t + j) * P:(t + j + 1) * P],
                                    ident_sb)
                            nc.scalar.copy(
                                out=quads[t // 4][:, :cnt,
                                                  qbl * P:(qbl + 1) * P],
                                in_=stg[:, :cnt, :])
                            t += cnt
                    # PV: ctxT[128, 512] accumulate over key tiles.
                    # Ragged key tiles only contribute to the query sub-blocks
                    # that cover them -> accumulate into a column slice; tile 0
                    # is covered by every sub-block, so start=True initializes
                    # the full bank.
                    tiles_per_qbl = [kns[i] // P for i in range(4)]
                    cps = ps_cx.tile([P, 512], F32, tag="ps_cx")
                    for t in range(ntile):
                        q0 = P * min(i for i in range(4)
                                     if tiles_per_qbl[i] > t)
                        nc.tensor.matmul(
                            cps[:, q0:], vnat[:, t, :],
                            quads[t // 4][:, t % 4, q0:],
                            start=(t == 0), stop=(t == ntile - 1))
                    cst = cs_pool.tile([P, 512], F32R, tag="cst")
                    nc.scalar.copy(out=cst, in_=cps)
                    nc.sync.dma_start(
                        out=ctx_sp[h, :, qg * 512:(qg + 1) * 512], in_=cst)

        ph12.close()  # free QKV + attention SBUF before dense phase

        # ---------------- phase 3: dense projection ----------------
        with ExitStack() as ph3:
            cx_pool = ph3.enter_context(tc.tile_pool(name="cx", bufs=1))
            wd_pool = ph3.enter_context(tc.tile_pool(name="wd", bufs=1))
            st_pool = ph3.enter_context(tc.tile_pool(name="ostage", bufs=2))
            ps3 = ph3.enter_context(
                tc.tile_pool(name="ps3", bufs=8, space="PSUM"))
            ctx_sb = [cx_pool.tile([P, S], F32R, tag=f"ctx_{hh}", name=f"ctx_{hh}")
                      for hh in range(HPC)]
            for hh in range(HPC):
                for sc in range(4):
                    nc.sync.dma_start(
                        out=ctx_sb[hh][:, sc * 512:(sc + 1) * 512],
                        in_=ctx_sp[hh, :, sc * 512:(sc + 1) * 512])
            wdt = wd_pool.tile([P, HPC, H], F32R, tag="wdt")
            for oc in range(8):
                for hh in range(HPC):
                    nc.sync.dma_start(
                        out=wdt[:, hh, oc * 512:(oc + 1) * 512],
                        in_=wdT[hh * P:(hh + 1) * P, oc * 512:(oc + 1) * 512])
            for qb in range(QB):
                pss = [ps3.tile([P, 512], F32, tag="ps3", name=f"ps3_{qb}_{i}")
                       for i in range(8)]
                for dc in range(HPC):
                    for oc in range(8):
                        nc.tensor.matmul(
                            pss[oc], ctx_sb[dc][:, qb * P:(qb + 1) * P],
                            wdt[:, dc, oc * 512:(oc + 1) * 512],
                            start=(dc == 0), stop=(dc == HPC - 1))
                stage = st_pool.tile([P, H], F32, tag="ostage")
                for oc in range(8):
                    nc.any.tensor_copy(
                        out=stage[:, oc * 512:(oc + 1) * 512], in_=pss[oc])
                nc.sync.dma_start(
                    out=out_t[qb * P:(qb + 1) * P, :], in_=stage)

    nc.compile()
    return nc


def _host_prep(hidden_states, alibi, attention_mask, w_qkv, b_qkv, w_dense):
    """Returns (kNq, in_maps) for the 8 cores."""
    hidden = np.asarray(hidden_states, np.float32).reshape(S, H)
    mask = np.asarray(attention_mask).reshape(S, S)
    alibi = np.asarray(alibi, np.float32).reshape(NH, S)
    w_qkv = np.asarray(w_qkv, np.float32)
    b_qkv = np.asarray(b_qkv, np.float32)
    w_dense = np.asarray(w_dense, np.float32)

    allowed = ~mask
    assert allowed.any(axis=1).all(), "fully-masked row"
    limit = S - np.argmax(allowed[:, ::-1], axis=1)      # last allowed + 1
    recon = np.arange(S)[None, :] >= limit[:, None]
    if not np.array_equal(mask, recon):
        raise NotImplementedError("mask is not suffix-structured")
    kNq, tri = [], np.zeros((QB, P, P), np.float32)
    col = np.arange(S)
    for qb in range(QB):
        lb = limit[qb * P:(qb + 1) * P]
        kN = int(math.ceil(lb.max() / P) * P)
        if lb.min() < kN - P:
            raise NotImplementedError("mask boundary spans >128 cols in block")
        kNq.append(kN)
        cc = col[kN - P:kN]
        tri[qb] = np.where(cc[None, :] >= lb[:, None], NEG, 0.0)
    if any(kNq[i] > kNq[i + 1] for i in range(QB - 1)):
        raise NotImplementedError("non-monotone key ranges")

    hT = np.ascontiguousarray(hidden.T)                  # [H, S]
    wr = w_qkv.reshape(NH, 3, HD, H)
    br = b_qkv.reshape(NH, 3, HD)
    ident = np.eye(P, dtype=np.float32)

    in_maps = []
    for c in range(NCORES):
        hs = slice(HPC * c, HPC * (c + 1))
        Wq = wr[hs, 0].reshape(DPC, H) * INV_NORM
        Wk = wr[hs, 1].reshape(DPC, H)
        Wv = wr[hs, 2].reshape(DPC, H)
        wqkvT_c = np.ascontiguousarray(
            np.concatenate([Wq, Wk, Wv], axis=0).T)      # [H, 1536]
        bq = br[hs, 0].reshape(-1) * INV_NORM
        bk = br[hs, 1].reshape(-1)
        bv = br[hs, 2].reshape(-1)
        bqkv_c = np.concatenate([bq, bk, bv]).reshape(FC, P)
        wdT_c = np.ascontiguousarray(
            w_dense[:, DPC * c:DPC * (c + 1)].T)         # [512, H]
        in_maps.append({
            "hT": hT, "wqkvT": wqkvT_c, "bqkv": bqkv_c,
            "alibi_c": np.ascontiguousarray(alibi[hs]),
            "tri": tri, "ident": ident, "wdT": wdT_c,
        })
    return tuple(kNq), in_maps


def kernel(hidden_states, residual, alibi, attention_mask,
           w_qkv, b_qkv, w_dense, b_dense):
    kNq, in_maps = _host_prep(hidden_states, alibi, attention_mask,
                              w_qkv, b_qkv, w_dense)
    if kNq not in _CACHE:
        _CACHE[kNq] = _build(kNq)
    nc = _CACHE[kNq]
    res = run_bass_kernel_spmd(nc, in_maps, list(range(NCORES)))
    acc = res.results[0]["out_part"].astype(np.float64)
    for c in range(1, NCORES):
        acc += res.results[c]["out_part"]
    out = acc.astype(np.float32) + np.asarray(b_dense, np.float32)[None, :]
    out = out + np.asarray(residual, np.float32).reshape(S, H)
    return out.reshape(B, S, H).astype(np.float32)


# revision 19
# speedup vs baseline: 20201.3568x; 1.1202x over previous
"""BloomAttention (B=1, S=2048, H=4096, NH=32) on 8 Trainium2 cores.

Megatron-style tensor parallelism over heads: each core owns 4 heads.
 - QKV projection: column-parallel (each core computes its heads' Q/K/V)
 - attention: fully local per core (head-parallel)
 - dense projection: row-parallel -> per-core partial outputs, summed on host

All matmuls run in float32r (TF32-like, full PE speed at moving dim >=256).
Layouts keep the contraction dim on SBUF partitions:
   hiddenT [H, S], w_qkvT [H, 3*512], QT/KT/VT per head [128, S],
   probsT [keys, q], ctxT [128, S], w_denseT [512, H].
Causal structure is exploited by truncating each 128-query block's key range;
the diagonal 128x128 block is masked with a host-provided additive tile.
"""
import math
import numpy as np
from contextlib import ExitStack

import concourse.bacc as bacc
import concourse.bass as bass
import concourse.mybir as mybir
import concourse.tile as tile
from concourse.bass_utils import run_bass_kernel_spmd

# problem dims (hardcoded per contract)
B, S, H, NH = 1, 2048, 4096, 32
HD = H // NH            # 128
NCORES = 8
HPC = NH // NCORES      # 4 heads per core
DPC = HPC * HD          # 512 features per core
FC = 3 * HPC            # 12 feature chunks of 128 in QKV output
INV_NORM = 1.0 / math.sqrt(HD)
NEG = float(np.finfo(np.float32).min)
P = 128
QB = S // P             # 16 query blocks
F32 = mybir.dt.float32
F32R = mybir.dt.float32r

_CACHE = {}


def _build(kNq):
    """Build the SPMD program for one core. kNq[qb] = key columns needed for
    query block qb (multiple of 128). Returns compiled Bacc."""
    nc = bacc.Bacc("TRN2", target_bir_lowering=False, debug=False,
                   num_devices=NCORES)

    hT = nc.dram_tensor("hT", [H, S], F32R, kind="ExternalInput")
    wqkvT = nc.dram_tensor("wqkvT", [H, FC * P], F32R, kind="ExternalInput")
    bqkv = nc.dram_tensor("bqkv", [FC, P], F32, kind="ExternalInput")
    alibi_t = nc.dram_tensor("alibi_c", [HPC, S], F32, kind="ExternalInput")
    tri_t = nc.dram_tensor("tri", [QB, P, P], F32, kind="ExternalInput")
    ident_t = nc.dram_tensor("ident", [P, P], F32R, kind="ExternalInput")
    wdT = nc.dram_tensor("wdT", [DPC, H], F32R, kind="ExternalInput")
    ctx_sp = nc.dram_tensor("ctx_spill", [HPC, P, S], F32R)
    out_t = nc.dram_tensor("out_part", [S, H], F32, kind="ExternalOutput")

    KP = 8                      # contraction panels of 512 rows
    JP = H // KP // P           # 4 h-chunks per panel

    with tile.TileContext(nc) as tc, ExitStack() as top:
        singles = top.enter_context(tc.tile_pool(name="singles", bufs=1))
        ph12 = top.enter_context(ExitStack())
        qkv_pool = ph12.enter_context(tc.tile_pool(name="qkv", bufs=1))
        # persistent QT/KT tiles per head [128, S] (head dim on partitions)
        qk_tiles = [[qkv_pool.tile([P, S], F32R, tag=f"qkv_{c}_{h}",
                                   name=f"qkv_{c}_{h}")
                     for h in range(HPC)] for c in range(2)]
        # V in natural layout: per key-tile sc -> [128 keys, 512 hd]
        v_tiles = [qkv_pool.tile([P, DPC], F32R, tag=f"v_{sc}",
                                 name=f"v_{sc}")
                   for sc in range(S // P)]
        ident_sb = singles.tile([P, P], F32R, tag="ident")
        nc.sync.dma_start(out=ident_sb, in_=ident_t[:, :])
        bias_sb = singles.tile([P, FC], F32, tag="bias")
        nc.sync.dma_start(
            out=bias_sb,
            in_=bass.AP(tensor=bqkv, offset=0, ap=[[1, P], [P, FC]]))
        # V bias broadcast to all partitions: bqkv rows 8..11 flattened [512]
        bv_bc = singles.tile([P, DPC], F32, tag="bv_bc")
        nc.gpsimd.dma_start(
            out=bv_bc,
            in_=bass.AP(tensor=bqkv, offset=2 * HPC * P,
                        ap=[[0, P], [1, DPC]]))

        # ---------------- phase 1: QKV projection ----------------
        with ExitStack() as ph1:
            hid_pool = ph1.enter_context(tc.tile_pool(name="hid", bufs=2))
            wq_pool = ph1.enter_context(tc.tile_pool(name="wq", bufs=3))
            ps1 = ph1.enter_context(
                tc.tile_pool(name="ps1", bufs=8, space="PSUM"))
            for kp in range(KP):
                hp = hid_pool.tile([P, JP, S], F32R, tag="hp")
                for j in range(JP):
                    r0 = (kp * JP + j) * P
                    nc.sync.dma_start(out=hp[:, j, :], in_=hT[r0:r0 + P, :])
                # Q and K: feature chunks on partitions
                for fc in range(2 * HPC):
                    wt = wq_pool.tile([P, JP, P], F32R, tag="wt")
                    nc.sync.dma_start(
                        out=wt,
                        in_=wqkvT[kp * JP * P:(kp + 1) * JP * P,
                                  fc * P:(fc + 1) * P].rearrange(
                                      "(j p) f -> p j f", p=P))
                    comp, head = fc // HPC, fc % HPC
                    dest = qk_tiles[comp][head]
                    for sb4 in range(S // 512):
                        ps = ps1.tile([P, 512], F32, tag="ps1")
                        for j in range(JP):
                            nc.tensor.matmul(
                                ps, wt[:, j, :],
                                hp[:, j, sb4 * 512:(sb4 + 1) * 512],
                                start=(j == 0), stop=(j == JP - 1))
                        dsl = dest[:, sb4 * 512:(sb4 + 1) * 512]
                        if kp == 0:
                            nc.scalar.activation(
                                out=dsl, in_=ps,
                                func=mybir.ActivationFunctionType.Identity,
                                bias=bias_sb[:, fc:fc + 1], scale=1.0)
                        else:
                            nc.vector.tensor_add(
                                out=dsl, in0=ps, in1=dsl)
                # V: natural layout, hidden chunks stationary, wv moving
                wv = wq_pool.tile([P, JP, DPC], F32R, tag="wv")
                nc.sync.dma_start(
                    out=wv,
                    in_=wqkvT[kp * JP * P:(kp + 1) * JP * P,
                              2 * HPC * P:].rearrange(
                                  "(j p) f -> p j f", p=P))
                for sc in range(S // P):
                    ps = ps1.tile([P, 512], F32, tag="ps1")
                    for j in range(JP):
                        nc.tensor.matmul(
                            ps, hp[:, j, sc * P:(sc + 1) * P],
                            wv[:, j, :],
                            start=(j == 0), stop=(j == JP - 1))
                    if kp == 0:
                        nc.scalar.copy(out=v_tiles[sc], in_=ps)
                    else:
                        nc.vector.tensor_add(
                            out=v_tiles[sc], in0=ps, in1=v_tiles[sc])
                if kp == KP - 1:
                    # fold in the V bias (broadcast row over partitions)
                    for sc in range(S // P):
                        nc.vector.tensor_add(
                            out=v_tiles[sc], in0=v_tiles[sc], in1=bv_bc)

        # ---------------- phase 2: attention ----------------
        with ExitStack() as ph2:
            al_pool = ph2.enter_context(tc.tile_pool(name="alibi", bufs=1))
            sr_pool = ph2.enter_context(tc.tile_pool(name="srow", bufs=3))
            pr_pool = ph2.enter_context(tc.tile_pool(name="prow", bufs=2))
            pq_pool = ph2.enter_context(tc.tile_pool(name="pquad", bufs=5))
            tri_pool = ph2.enter_context(tc.tile_pool(name="tri", bufs=2))
            sm_pool = ph2.enter_context(tc.tile_pool(name="small", bufs=8))
            cs_pool = ph2.enter_context(tc.tile_pool(name="ctxstage", bufs=2))
            ps_sc = ph2.enter_context(
                tc.tile_pool(name="ps_sc", bufs=3, space="PSUM"))
            ps_st = ph2.enter_context(
                tc.tile_pool(name="ps_st", bufs=3, space="PSUM"))
            ps_cx = ph2.enter_context(
                tc.tile_pool(name="ps_cx", bufs=2, space="PSUM"))

            NG = QB // 4        # 4 query groups of 512
            for h in range(HPC):
                qt, kt = qk_tiles[0][h], qk_tiles[1][h]
                alb = al_pool.tile([P, S], F32, tag="alb")
                nc.gpsimd.dma_start(
                    out=alb,
                    in_=bass.AP(tensor=alibi_t, offset=h * S,
                                ap=[[0, P], [1, S]]))

                for qg in range(NG):
                    kns = [kNq[4 * qg + i] for i in range(4)]
                    ntile = max(kns) // P
                    nquad = (ntile + 3) // 4
                    quads = [pq_pool.tile([P, 4, 512], F32R, tag="pquad",
                                          name=f"pq_{h}_{qg}_{a}")
                             for a in range(nquad)]

                    for qbl in range(4):
                        qb = 4 * qg + qbl
                        kN = kNq[qb]
                        nch = (kN + 511) // 512
                        srow = sr_pool.tile([P, S], F32, tag="srow")
                        prow = pr_pool.tile([P, S], F32R, tag="prow")
                        nbias = sm_pool.tile([P, 1], F32, tag="nbias")
                        tot = sm_pool.tile([P, 1], F32, tag="tot")
                        rinv = sm_pool.tile([P, 1], F32, tag="rinv")
                        tri_sb = tri_pool.tile([P, P], F32, tag="tri")
                        nc.sync.dma_start(out=tri_sb, in_=tri_t[qb])
                        for kc in range(nch):
                            N = min(512, kN - 512 * kc)
                            ps = ps_sc.tile([P, 512], F32, tag="ps_sc")
                            nc.tensor.matmul(
                                ps[:, :N], qt[:, qb * P:(qb + 1) * P],
                                kt[:, kc * 512:kc * 512 + N],
                                start=True, stop=True)
                            # scores + alibi -> SBUF (frees the PSUM bank)
                            nc.vector.tensor_add(
                                out=srow[:, kc * 512:kc * 512 + N],
                                in0=ps[:, :N],
                                in1=alb[:, kc * 512:kc * 512 + N])
                        nc.vector.tensor_add(
                            out=srow[:, kN - P:kN], in0=srow[:, kN - P:kN],
                            in1=tri_sb)
                        nc.vector.tensor_reduce(
                            out=nbias, in_=srow[:, :kN],
                            op=mybir.AluOpType.max,
                            axis=mybir.AxisListType.X, negate=True)
                        nc.scalar.activation(
                            out=prow[:, :kN], in_=srow[:, :kN],
                            func=mybir.ActivationFunctionType.Exp,
                            bias=nbias, scale=1.0, accum_out=tot)
                        nc.vector.reciprocal(out=rinv, in_=tot)
                        nc.vector.tensor_scalar_mul(
                            out=prow[:, :kN], in0=prow[:, :kN], scalar1=rinv)
                        # transpose probs into key-major quads
                        # quad tile layout: [P, tile_in_quad(4), q(512)]
                        ntile_q = kN // P
                        t = 0
                        while t < ntile_q:
                            cnt = min(4, ntile_q - t)
                            stg = ps_st.tile([P, 4, P], F32R, tag="stg")
                            for j in range(cnt):
                                nc.tensor.transpose(
                                    stg[:, j, :],
                                    prow[:, (t + j) * P:(t + j + 1) * P],
                                    ident_sb)
                            nc.scalar.copy(
                                out=quads[t // 4][:, :cnt,
                                                  qbl * P:(qbl + 1) * P],
                                in_=stg[:, :cnt, :])
                            t += cnt
                    # PV: ctxT[128, 512] accumulate over key tiles.
                    # Ragged key tiles only contribute to the query sub-blocks
                    # that cover them -> accumulate into a column slice; tile 0
                    # is covered by every sub-block, so start=True initializes
                    # the full bank.
                    tiles_per_qbl = [kns[i] // P for i in range(4)]
                    cps = ps_cx.tile([P, 512], F32, tag="ps_cx")
                    for t in range(ntile):
                        q0 = P * min(i for i in range(4)
                                     if tiles_per_qbl[i] > t)
                        nc.tensor.matmul(
                            cps[:, q0:], v_tiles[t][:, h * P:(h + 1) * P],
                            quads[t // 4][:, t % 4, q0:],
                            start=(t == 0), stop=(t == ntile - 1))
                    cst = cs_pool.tile([P, 512], F32R, tag="cst")
                    nc.scalar.copy(out=cst, in_=cps)
                    nc.sync.dma_start(
                        out=ctx_sp[h, :, qg * 512:(qg + 1) * 512], in_=cst)

        ph12.close()  # free QKV + attention SBUF before dense phase

        # ---------------- phase 3: dense projection ----------------
        with ExitStack() as ph3:
            cx_pool = ph3.enter_context(tc.tile_pool(name="cx", bufs=1))
            wd_pool = ph3.enter_context(tc.tile_pool(name="wd", bufs=1))
            st_pool = ph3.enter_context(tc.tile_pool(name="ostage", bufs=2))
            ps3 = ph3.enter_context(
                tc.tile_pool(name="ps3", bufs=8, space="PSUM"))
            ctx_sb = [cx_pool.tile([P, S], F32R, tag=f"ctx_{hh}", name=f"ctx_{hh}")
                      for hh in range(HPC)]
            for hh in range(HPC):
                for sc in range(4):
                    nc.sync.dma_start(
                        out=ctx_sb[hh][:, sc * 512:(sc + 1) * 512],
                        in_=ctx_sp[hh, :, sc * 512:(sc + 1) * 512])
            wdt = wd_pool.tile([P, HPC, H], F32R, tag="wdt")
            for oc in range(8):
                for hh in range(HPC):
                    nc.sync.dma_start(
                        out=wdt[:, hh, oc * 512:(oc + 1) * 512],
                        in_=wdT[hh * P:(hh + 1) * P, oc * 512:(oc + 1) * 512])
            for qb in range(QB):
                pss = [ps3.tile([P, 512], F32, tag="ps3", name=f"ps3_{qb}_{i}")
                       for i in range(8)]
                for dc in range(HPC):
                    for oc in range(8):
                        nc.tensor.matmul(
                            pss[oc], ctx_sb[dc][:, qb * P:(qb + 1) * P],
                            wdt[:, dc, oc * 512:(oc + 1) * 512],
                            start=(dc == 0), stop=(dc == HPC - 1))
                stage = st_pool.tile([P, H], F32, tag="ostage")
                for oc in range(8):
                    nc.any.tensor_copy(
                        out=stage[:, oc * 512:(oc + 1) * 512], in_=pss[oc])
                nc.sync.dma_start(
                    out=out_t[qb * P:(qb + 1) * P, :], in_=stage)

    nc.compile()
    return nc


def _host_prep(hidden_states, alibi, attention_mask, w_qkv, b_qkv, w_dense):
    """Returns (kNq, in_maps) for the 8 cores."""
    hidden = np.asarray(hidden_states, np.float32).reshape(S, H)
    mask = np.asarray(attention_mask).reshape(S, S)
    alibi = np.asarray(alibi, np.float32).reshape(NH, S)
    w_qkv = np.asarray(w_qkv, np.float32)
    b_qkv = np.asarray(b_qkv, np.float32)
    w_dense = np.asarray(w_dense, np.float32)

    allowed = ~mask
    assert allowed.any(axis=1).all(), "fully-masked row"
    limit = S - np.argmax(allowed[:, ::-1], axis=1)      # last allowed + 1
    recon = np.arange(S)[None, :] >= limit[:, None]
    if not np.array_equal(mask, recon):
        raise NotImplementedError("mask is not suffix-structured")
    kNq, tri = [], np.zeros((QB, P, P), np.float32)
    col = np.arange(S)
    for qb in range(QB):
        lb = limit[qb * P:(qb + 1) * P]
        kN = int(math.ceil(lb.max() / P) * P)
        if lb.min() < kN - P:
            raise NotImplementedError("mask boundary spans >128 cols in block")
        kNq.append(kN)
        cc = col[kN - P:kN]
        tri[qb] = np.where(cc[None, :] >= lb[:, None], NEG, 0.0)
    if any(kNq[i] > kNq[i + 1] for i in range(QB - 1)):
        raise NotImplementedError("non-monotone key ranges")

    hT = np.ascontiguousarray(hidden.T)                  # [H, S]
    wr = w_qkv.reshape(NH, 3, HD, H)
    br = b_qkv.reshape(NH, 3, HD)
    ident = np.eye(P, dtype=np.float32)

    in_maps = []
    for c in range(NCORES):
        hs = slice(HPC * c, HPC * (c + 1))
        Wq = wr[hs, 0].reshape(DPC, H) * INV_NORM
        Wk = wr[hs, 1].reshape(DPC, H)
        Wv = wr[hs, 2].reshape(DPC, H)
        wqkvT_c = np.ascontiguousarray(
            np.concatenate([Wq, Wk, Wv], axis=0).T)      # [H, 1536]
        bq = br[hs, 0].reshape(-1) * INV_NORM
        bk = br[hs, 1].reshape(-1)
        bv = br[hs, 2].reshape(-1)
        bqkv_c = np.concatenate([bq, bk, bv]).reshape(FC, P)
        wdT_c = np.ascontiguousarray(
            w_dense[:, DPC * c:DPC * (c + 1)].T)         # [512, H]
        in_maps.append({
            "hT": hT, "wqkvT": wqkvT_c, "bqkv": bqkv_c,
            "alibi_c": np.ascontiguousarray(alibi[hs]),
            "tri": tri, "ident": ident, "wdT": wdT_c,
        })
    return tuple(kNq), in_maps


def kernel(hidden_states, residual, alibi, attention_mask,
           w_qkv, b_qkv, w_dense, b_dense):
    kNq, in_maps = _host_prep(hidden_states, alibi, attention_mask,
                              w_qkv, b_qkv, w_dense)
    if kNq not in _CACHE:
        _CACHE[kNq] = _build(kNq)
    nc = _CACHE[kNq]
    res = run_bass_kernel_spmd(nc, in_maps, list(range(NCORES)))
    acc = res.results[0]["out_part"].astype(np.float64)
    for c in range(1, NCORES):
        acc += res.results[c]["out_part"]
    out = acc.astype(np.float32) + np.asarray(b_dense, np.float32)[None, :]
    out = out + np.asarray(residual, np.float32).reshape(S, H)
    return out.reshape(B, S, H).astype(np.float32)


# revision 26
# speedup vs baseline: 21928.6137x; 1.0855x over previous
"""BloomAttention (B=1, S=2048, H=4096, NH=32) on 8 Trainium2 cores.

Megatron-style tensor parallelism over heads: each core owns 4 heads.
 - QKV projection: column-parallel (each core computes its heads' Q/K/V)
 - attention: fully local per core (head-parallel)
 - dense projection: row-parallel -> per-core partial outputs, summed on host

All matmuls run in float32r (TF32-like, full PE speed at moving dim >=256).
Layouts keep the contraction dim on SBUF partitions:
   hiddenT [H, S], w_qkvT [H, 3*512], QT/KT/VT per head [128, S],
   probsT [keys, q], ctxT [128, S], w_denseT [512, H].
Causal structure is exploited by truncating each 128-query block's key range;
the diagonal 128x128 block is masked with a host-provided additive tile.
"""
import math
import numpy as np
from contextlib import ExitStack

import concourse.bacc as bacc
import concourse.bass as bass
import concourse.mybir as mybir
import concourse.tile as tile
from concourse.bass_utils import run_bass_kernel_spmd

# problem dims (hardcoded per contract)
B, S, H, NH = 1, 2048, 4096, 32
HD = H // NH            # 128
NCORES = 8
HPC = NH // NCORES      # 4 heads per core
DPC = HPC * HD          # 512 features per core
FC = 3 * HPC            # 12 feature chunks of 128 in QKV output
INV_NORM = 1.0 / math.sqrt(HD)
NEG = float(np.finfo(np.float32).min)
P = 128
QB = S // P             # 16 query blocks
F32 = mybir.dt.float32
F32R = mybir.dt.float32r

_CACHE = {}


def _build(kNq):
    """Build the SPMD program for one core. kNq[qb] = key columns needed for
    query block qb (multiple of 128). Returns compiled Bacc."""
    nc = bacc.Bacc("TRN2", target_bir_lowering=False, debug=False,
                   num_devices=NCORES)

    hT = nc.dram_tensor("hT", [H, S], F32R, kind="ExternalInput")
    wqkvT = nc.dram_tensor("wqkvT", [H, FC * P], F32R, kind="ExternalInput")
    bqkv = nc.dram_tensor("bqkv", [FC, P], F32, kind="ExternalInput")
    alibi_t = nc.dram_tensor("alibi_c", [HPC, S], F32, kind="ExternalInput")
    tri_t = nc.dram_tensor("tri", [QB, P, P], F32, kind="ExternalInput")
    ident_t = nc.dram_tensor("ident", [P, P], F32R, kind="ExternalInput")
    wdT = nc.dram_tensor("wdT", [DPC, H], F32R, kind="ExternalInput")
    ctx_sp = nc.dram_tensor("ctx_spill", [HPC, P, S], F32R)
    out_t = nc.dram_tensor("out_part", [S, H], F32, kind="ExternalOutput")

    KP = 8                      # contraction panels of 512 rows
    JP = H // KP // P           # 4 h-chunks per panel

    with tile.TileContext(nc) as tc, ExitStack() as top:
        singles = top.enter_context(tc.tile_pool(name="singles", bufs=1))
        ph12 = top.enter_context(ExitStack())
        qkv_pool = ph12.enter_context(tc.tile_pool(name="qkv", bufs=1))
        # persistent QT/KT tiles per head [128, S] (head dim on partitions)
        qk_tiles = [[qkv_pool.tile([P, S], F32R, tag=f"qkv_{c}_{h}",
                                   name=f"qkv_{c}_{h}")
                     for h in range(HPC)] for c in range(2)]
        # V in natural layout: per key-tile sc -> [128 keys, 512 hd]
        v_tiles = [qkv_pool.tile([P, DPC], F32R, tag=f"v_{sc}",
                                 name=f"v_{sc}")
                   for sc in range(S // P)]
        ident_sb = singles.tile([P, P], F32R, tag="ident")
        nc.sync.dma_start(out=ident_sb, in_=ident_t[:, :])
        bias_sb = singles.tile([P, FC], F32, tag="bias")
        nc.sync.dma_start(
            out=bias_sb,
            in_=bass.AP(tensor=bqkv, offset=0, ap=[[1, P], [P, FC]]))
        # V bias broadcast to all partitions: bqkv rows 8..11 flattened [512]
        bv_bc = singles.tile([P, DPC], F32, tag="bv_bc")
        nc.gpsimd.dma_start(
            out=bv_bc,
            in_=bass.AP(tensor=bqkv, offset=2 * HPC * P,
                        ap=[[0, P], [1, DPC]]))

        # ---------------- phase 1: QKV projection ----------------
        with ExitStack() as ph1:
            hid_pool = ph1.enter_context(tc.tile_pool(name="hid", bufs=2))
            wq_pool = ph1.enter_context(tc.tile_pool(name="wq", bufs=3))
            ps1 = ph1.enter_context(
                tc.tile_pool(name="ps1", bufs=4, space="PSUM"))
            for kp in range(KP):
                hp = hid_pool.tile([P, JP, S], F32R, tag="hp")
                for j in range(JP):
                    r0 = (kp * JP + j) * P
                    nc.sync.dma_start(out=hp[:, j, :], in_=hT[r0:r0 + P, :])
                # Q and K: feature chunks on partitions
                for fc in range(2 * HPC):
                    wt = wq_pool.tile([P, JP, P], F32R, tag="wt")
                    nc.sync.dma_start(
                        out=wt,
                        in_=wqkvT[kp * JP * P:(kp + 1) * JP * P,
                                  fc * P:(fc + 1) * P].rearrange(
                                      "(j p) f -> p j f", p=P))
                    comp, head = fc // HPC, fc % HPC
                    dest = qk_tiles[comp][head]
                    for sb2 in range(S // 1024):
                        ps = ps1.tile([P, 1024], F32, tag="ps1")
                        for half in range(2):
                            for j in range(JP):
                                nc.tensor.matmul(
                                    ps[:, half * 512:(half + 1) * 512],
                                    wt[:, j, :],
                                    hp[:, j, sb2 * 1024 + half * 512:
                                       sb2 * 1024 + (half + 1) * 512],
                                    start=(j == 0), stop=(j == JP - 1))
                        dsl = dest[:, sb2 * 1024:(sb2 + 1) * 1024]
                        if kp == 0:
                            nc.scalar.activation(
                                out=dsl, in_=ps,
                                func=mybir.ActivationFunctionType.Identity,
                                bias=bias_sb[:, fc:fc + 1], scale=1.0)
                        else:
                            nc.vector.tensor_add(
                                out=dsl, in0=ps, in1=dsl)
                # V: natural layout, hidden chunks stationary, wv moving
                wv = wq_pool.tile([P, JP, DPC], F32R, tag="wv")
                nc.sync.dma_start(
                    out=wv,
                    in_=wqkvT[kp * JP * P:(kp + 1) * JP * P,
                              2 * HPC * P:].rearrange(
                                  "(j p) f -> p j f", p=P))
                for sc2 in range(S // P // 2):
                    ps = ps1.tile([P, 1024], F32, tag="ps1")
                    for half in range(2):
                        sc = 2 * sc2 + half
                        for j in range(JP):
                            nc.tensor.matmul(
                                ps[:, half * 512:(half + 1) * 512],
                                hp[:, j, sc * P:(sc + 1) * P],
                                wv[:, j, :],
                                start=(j == 0), stop=(j == JP - 1))
                    # drain both halves; v tiles are per key-tile [128, 512]
                    for half in range(2):
                        sc = 2 * sc2 + half
                        psl = ps[:, half * 512:(half + 1) * 512]
                        if kp == 0:
                            nc.scalar.copy(out=v_tiles[sc], in_=psl)
                        else:
                            nc.vector.tensor_add(
                                out=v_tiles[sc], in0=psl, in1=v_tiles[sc])
                if kp == KP - 1:
                    # fold in the V bias (broadcast row over partitions)
                    for sc in range(S // P):
                        nc.vector.tensor_add(
                            out=v_tiles[sc], in0=v_tiles[sc], in1=bv_bc)

        # ---------------- phase 2: attention ----------------
        with ExitStack() as ph2:
            al_pool = ph2.enter_context(tc.tile_pool(name="alibi", bufs=2))
            sr_pool = ph2.enter_context(tc.tile_pool(name="srow", bufs=3))
            pr_pool = ph2.enter_context(tc.tile_pool(name="prow", bufs=2))
            pq_pool = ph2.enter_context(tc.tile_pool(name="pquad", bufs=4))
            tri_pool = ph2.enter_context(tc.tile_pool(name="tri", bufs=2))
            sm_pool = ph2.enter_context(tc.tile_pool(name="small", bufs=8))
            cs_pool = ph2.enter_context(tc.tile_pool(name="ctxstage", bufs=2))
            ps_sc = ph2.enter_context(
                tc.tile_pool(name="ps_sc", bufs=3, space="PSUM"))
            ps_st = ph2.enter_context(
                tc.tile_pool(name="ps_st", bufs=3, space="PSUM"))
            ps_cx = ph2.enter_context(
                tc.tile_pool(name="ps_cx", bufs=2, space="PSUM"))

            NG = QB // 4        # 4 query groups of 512
            for h in range(HPC):
                qt, kt = qk_tiles[0][h], qk_tiles[1][h]
                alb = al_pool.tile([P, S], F32, tag="alb")
                nc.gpsimd.dma_start(
                    out=alb,
                    in_=bass.AP(tensor=alibi_t, offset=h * S,
                                ap=[[0, P], [1, S]]))

                for qg in range(NG):
                    kns = [kNq[4 * qg + i] for i in range(4)]
                    ntile = max(kns) // P
                    nquad = (ntile + 3) // 4
                    quads = [pq_pool.tile([P, 4, 512], F32R, tag="pquad",
                                          name=f"pq_{h}_{qg}_{a}")
                             for a in range(nquad)]

                    for qbl in range(4):
                        qb = 4 * qg + qbl
                        kN = kNq[qb]
                        nch = (kN + 511) // 512
                        srow = sr_pool.tile([P, S], F32, tag="srow")
                        prow = pr_pool.tile([P, S], F32R, tag="prow")
                        nbias = sm_pool.tile([P, 1], F32, tag="nbias")
                        tot = sm_pool.tile([P, 1], F32, tag="tot")
                        rinv = sm_pool.tile([P, 1], F32, tag="rinv")
                        tri_sb = tri_pool.tile([P, P], F32, tag="tri")
                        nc.sync.dma_start(out=tri_sb, in_=tri_t[qb])
                        for kc in range(nch):
                            N = min(512, kN - 512 * kc)
                            ps = ps_sc.tile([P, 512], F32, tag="ps_sc")
                            nc.tensor.matmul(
                                ps[:, :N], qt[:, qb * P:(qb + 1) * P],
                                kt[:, kc * 512:kc * 512 + N],
                                start=True, stop=True)
                            # scores + alibi -> SBUF (frees the PSUM bank)
                            nc.vector.tensor_add(
                                out=srow[:, kc * 512:kc * 512 + N],
                                in0=ps[:, :N],
                                in1=alb[:, kc * 512:kc * 512 + N])
                        nc.vector.tensor_add(
                            out=srow[:, kN - P:kN], in0=srow[:, kN - P:kN],
                            in1=tri_sb)
                        nc.vector.tensor_reduce(
                            out=nbias, in_=srow[:, :kN],
                            op=mybir.AluOpType.max,
                            axis=mybir.AxisListType.X, negate=True)
                        nc.scalar.activation(
                            out=prow[:, :kN], in_=srow[:, :kN],
                            func=mybir.ActivationFunctionType.Exp,
                            bias=nbias, scale=1.0, accum_out=tot)
                        nc.vector.reciprocal(out=rinv, in_=tot)
                        nc.vector.tensor_scalar_mul(
                            out=prow[:, :kN], in0=prow[:, :kN], scalar1=rinv)
                        # transpose probs into key-major quads
                        # quad tile layout: [P, tile_in_quad(4), q(512)]
                        ntile_q = kN // P
                        t = 0
                        while t < ntile_q:
                            cnt = min(4, ntile_q - t)
                            stg = ps_st.tile([P, 4, P], F32R, tag="stg")
                            for j in range(cnt):
                                nc.tensor.transpose(
                                    stg[:, j, :],
                                    prow[:, (t + j) * P:(t + j + 1) * P],
                                    ident_sb)
                            nc.scalar.copy(
                                out=quads[t // 4][:, :cnt,
                                                  qbl * P:(qbl + 1) * P],
                                in_=stg[:, :cnt, :])
                            t += cnt
                    # PV: ctxT[128, 512] accumulate over key tiles.
                    # Ragged key tiles only contribute to the query sub-blocks
                    # that cover them -> accumulate into a column slice; tile 0
                    # is covered by every sub-block, so start=True initializes
                    # the full bank.
                    tiles_per_qbl = [kns[i] // P for i in range(4)]
                    cps = ps_cx.tile([P, 512], F32, tag="ps_cx")
                    for t in range(ntile):
                        q0 = P * min(i for i in range(4)
                                     if tiles_per_qbl[i] > t)
                        nc.tensor.matmul(
                            cps[:, q0:], v_tiles[t][:, h * P:(h + 1) * P],
                            quads[t // 4][:, t % 4, q0:],
                            start=(t == 0), stop=(t == ntile - 1))
                    cst = cs_pool.tile([P, 512], F32R, tag="cst")
                    nc.scalar.copy(out=cst, in_=cps)
                    nc.sync.dma_start(
                        out=ctx_sp[h, :, qg * 512:(qg + 1) * 512], in_=cst)

        ph12.close()  # free QKV + attention SBUF before dense phase

        # ---------------- phase 3: dense projection ----------------
        with ExitStack() as ph3:
            cx_pool = ph3.enter_context(tc.tile_pool(name="cx", bufs=1))
            wd_pool = ph3.enter_context(tc.tile_pool(name="wd", bufs=1))
            st_pool = ph3.enter_context(tc.tile_pool(name="ostage", bufs=2))
            ps3 = ph3.enter_context(
                tc.tile_pool(name="ps3", bufs=8, space="PSUM"))
            ctx_sb = [cx_pool.tile([P, S], F32R, tag=f"ctx_{hh}", name=f"ctx_{hh}")
                      for hh in range(HPC)]
            for hh in range(HPC):
                for sc in range(4):
                    nc.sync.dma_start(
                        out=ctx_sb[hh][:, sc * 512:(sc + 1) * 512],
                        in_=ctx_sp[hh, :, sc * 512:(sc + 1) * 512])
            wdt = wd_pool.tile([P, HPC, H], F32R, tag="wdt")
            for oc in range(8):
                for hh in range(HPC):
                    nc.sync.dma_start(
                        out=wdt[:, hh, oc * 512:(oc + 1) * 512],
                        in_=wdT[hh * P:(hh + 1) * P, oc * 512:(oc + 1) * 512])
            for qb in range(QB):
                pss = [ps3.tile([P, 512], F32, tag="ps3", name=f"ps3_{qb}_{i}")
                       for i in range(8)]
                for oc in range(8):
                    for dc in range(HPC):
                        nc.tensor.matmul(
                            pss[oc], ctx_sb[dc][:, qb * P:(qb + 1) * P],
                            wdt[:, dc, oc * 512:(oc + 1) * 512],
                            start=(dc == 0), stop=(dc == HPC - 1))
                stage = st_pool.tile([P, H], F32, tag="ostage")
                for oc in range(8):
                    nc.any.tensor_copy(
                        out=stage[:, oc * 512:(oc + 1) * 512], in_=pss[oc])
                nc.sync.dma_start(
                    out=out_t[qb * P:(qb + 1) * P, :], in_=stage)

    nc.compile()
    return nc


def _host_prep(hidden_states, alibi, attention_mask, w_qkv, b_qkv, w_dense):
    """Returns (kNq, in_maps) for the 8 cores."""
    hidden = np.asarray(hidden_states, np.float32).reshape(S, H)
    mask = np.asarray(attention_mask).reshape(S, S)
    alibi = np.asarray(alibi, np.float32).reshape(NH, S)
    w_qkv = np.asarray(w_qkv, np.float32)
    b_qkv = np.asarray(b_qkv, np.float32)
    w_dense = np.asarray(w_dense, np.float32)

    allowed = ~mask
    assert allowed.any(axis=1).all(), "fully-masked row"
    limit = S - np.argmax(allowed[:, ::-1], axis=1)      # last allowed + 1
    recon = np.arange(S)[None, :] >= limit[:, None]
    if not np.array_equal(mask, recon):
        raise NotImplementedError("mask is not suffix-structured")
    kNq, tri = [], np.zeros((QB, P, P), np.float32)
    col = np.arange(S)
    for qb in range(QB):
        lb = limit[qb * P:(qb + 1) * P]
        kN = int(math.ceil(lb.max() / P) * P)
        if lb.min() < kN - P:
            raise NotImplementedError("mask boundary spans >128 cols in block")
        kNq.append(kN)
        cc = col[kN - P:kN]
        tri[qb] = np.where(cc[None, :] >= lb[:, None], NEG, 0.0)
    if any(kNq[i] > kNq[i + 1] for i in range(QB - 1)):
        raise NotImplementedError("non-monotone key ranges")

    hT = np.ascontiguousarray(hidden.T)                  # [H, S]
    wr = w_qkv.reshape(NH, 3, HD, H)
    br = b_qkv.reshape(NH, 3, HD)
    ident = np.eye(P, dtype=np.float32)

    in_maps = []
    for c in range(NCORES):
        hs = slice(HPC * c, HPC * (c + 1))
        Wq = wr[hs, 0].reshape(DPC, H) * INV_NORM
        Wk = wr[hs, 1].reshape(DPC, H)
        Wv = wr[hs, 2].reshape(DPC, H)
        wqkvT_c = np.ascontiguousarray(
            np.concatenate([Wq, Wk, Wv], axis=0).T)      # [H, 1536]
        bq = br[hs, 0].reshape(-1) * INV_NORM
        bk = br[hs, 1].reshape(-1)
        bv = br[hs, 2].reshape(-1)
        bqkv_c = np.concatenate([bq, bk, bv]).reshape(FC, P)
        wdT_c = np.ascontiguousarray(
            w_dense[:, DPC * c:DPC * (c + 1)].T)         # [512, H]
        in_maps.append({
            "hT": hT, "wqkvT": wqkvT_c, "bqkv": bqkv_c,
            "alibi_c": np.ascontiguousarray(alibi[hs]),
            "tri": tri, "ident": ident, "wdT": wdT_c,
        })
    return tuple(kNq), in_maps


def kernel(hidden_states, residual, alibi, attention_mask,
           w_qkv, b_qkv, w_dense, b_dense):
    kNq, in_maps = _host_prep(hidden_states, alibi, attention_mask,
                              w_qkv, b_qkv, w_dense)
    if kNq not in _CACHE:
        _CACHE[kNq] = _build(kNq)
    nc = _CACHE[kNq]
    res = run_bass_kernel_spmd(nc, in_maps, list(range(NCORES)))
    acc = res.results[0]["out_part"].astype(np.float64)
    for c in range(1, NCORES):
        acc += res.results[c]["out_part"]
    out = acc.astype(np.float32) + np.asarray(b_dense, np.float32)[None, :]
    out = out + np.asarray(residual, np.float32).reshape(S, H)
    return out.reshape(B, S, H).astype(np.float32)
